# revision 46
# baseline (speedup 1.0000x reference)
"""Box-from-mask kernel for Trainium2 (8 NeuronCores, SPMD data-parallel).

Problem: masks [100, 800, 1280] f32 -> boxes [100, 2, 2] f32 where
box[n] = [[xmin, ymin], [xmax, ymax]] of {(y, x) : masks[n, y, x] > 0.5},
with empty-mask sentinels xmin=W, ymin=H, xmax=-1, ymax=-1.

Flat-row sharding: the 100*800 = 80,000 mask rows are treated as one flat
[80000, 1280] array. Core c owns rows [c*10000, (c+1)*10000): 80 uniform
125-row tiles per core, no runt DMAs and zero duplicate traffic (125-row
tiles divide the row count exactly; engine time is set by the 1280-wide
free axis, so the 3 idle partitions are free).

Per-core device pipeline, per [125, 1280] block:
  - one DVE tensor_scalar(is_gt 0.5) -> 0/1 fp8 block, with accum_out(max)
    giving the per-row "any pixel" bit (one elementwise pass per element).
  - PE selector matmul (fp8 x fp8 -> fp32 PSUM, exact for 0/1 counts)
    accumulates per-column counts into three per-chunk [13, 512] PSUM
    tiles; the [125, 13] one-hot selector for each block routes every SBUF
    partition (= one mask row) to its mask's PSUM row, so blocks that
    straddle a mask boundary need no special casing.
Blocks arrive in partition-major DMA groups (9 tiles -> one 5.8 MB DMA,
each partition reading 46 KB contiguous) alternating between the SP and
ACT HWDGE rings; the final groups taper (4+2+1+1 tiles, all on SP so they
complete in FIFO order). The PSUM chains stop one block early: the final
block skips the PE and ships its binarized fp8 tile raw (160 KB), so the
counts convert+flush hides under the last transfer and the host folds the
final block's columns in. Outputs are the row-any bits [125, 80], column
presence [13, 1280], and the final fp8 tile; the min/max index arithmetic
happens host-side (exact).
"""

import sys

for _p in ("/opt/trn_rl_repo", "/opt/pypackages"):
    if _p not in sys.path:
        sys.path.append(_p)

import ml_dtypes
import numpy as np

import concourse.tile as tile
from concourse import bacc, mybir
from concourse.bass_utils import run_bass_kernel_spmd

N, H, W = 100, 800, 1280
N_CORES = 8
THRESHOLD = 0.5

ROWS = N * H  # 80,000 flat rows
P = 125  # rows per tile: 80,000 = 8 cores * 80 tiles * 125 rows, exactly.
# 128-row tiles would need 632 tiles for a uniform 8-way split (625 real),
# i.e. 7 tiles of duplicate traffic; 125-row tiles split evenly with zero
# overlap, and engine time is set by the 1280-wide free axis, not the
# partition count, so the 3 idle partitions cost nothing.
SHARD_ROWS = ROWS // N_CORES  # 10,000
NB = SHARD_ROWS // P  # 80 blocks of 125 rows
GSZ = 13  # max distinct masks touched by one core's shard
# DMA groups: sizes of consecutive tile groups fetched by one DMA each.
# Tapered tail: the short final groups shrink the serial compute left
# after the last transfer lands.
GROUP_SIZES = [9] * 8 + [4, 2, 1, 1]
assert sum(GROUP_SIZES) == NB

fp32 = mybir.dt.float32
fp16 = mybir.dt.float16
bf16 = mybir.dt.bfloat16
fp8 = mybir.dt.float8e4
Op = mybir.AluOpType


def _chunks(w):
    return [(c, min(512, w - c)) for c in range(0, w, 512)]


def _groups():
    """[(row_offset, n_tiles)] per DMA group."""
    out, r = [], 0
    for t in GROUP_SIZES:
        out.append((r, t))
        r += t * P
    return out


def _local_rows():
    """local_rows[p, B] = shard-local row held by partition p for block B.

    Partition-major DMA: group (R, T) lands rows R + p*T + a on partition p,
    column-block a.
    """
    cols = []
    for R, T in _groups():
        for a in range(T):
            cols.append(R + np.arange(P) * T + a)
    return np.stack(cols, axis=1)  # [P, NB]


LOCAL_ROWS = _local_rows()

RAW_BUFS = 4
BIN_BUFS = 6


def build_program():
    """One-core Bass/Tile program; run SPMD on all 8 cores."""
    chunks = _chunks(W)
    groups = _groups()
    tmax = max(t for _, t in groups)

    nc = bacc.Bacc(
        "TRN2", target_bir_lowering=False, debug=False, enable_asserts=False
    )
    masks = nc.dram_tensor("masks", [SHARD_ROWS, W], fp32, kind="ExternalInput").ap()
    # fp8 halves the selector's HBM traffic and the binarized tiles' SBUF
    # footprint; 0/1 are exact in fp8e4 and the fp8xfp8 matmul accumulates
    # in fp32 PSUM, so everything stays exact
    sel = nc.dram_tensor("sel", [128, NB * GSZ], fp8, kind="ExternalInput").ap()
    # NB+1 columns: the final block's two half-width binarizes accumulate
    # into separate columns (host maxes them)
    rowany_out = nc.dram_tensor(
        "rowany_out", [128, NB + 1], fp32, kind="ExternalOutput"
    ).ap()
    counts_out = nc.dram_tensor(
        "counts_out", [GSZ, W], fp16, kind="ExternalOutput"
    ).ap()
    # the final block's binarized tile, shipped raw: its column reduction
    # happens host-side, so the PSUM chains stop a block early and their
    # convert+flush hides under the last transfer instead of trailing it
    blast_out = nc.dram_tensor("blast_out", [P, W], fp8, kind="ExternalOutput").ap()

    with tile.TileContext(nc) as tc:
        with (
            tc.tile_pool(name="raw", bufs=RAW_BUFS) as rawp,
            tc.tile_pool(name="bin", bufs=BIN_BUFS) as binp,
            tc.tile_pool(name="consts", bufs=1) as constp,
            tc.tile_pool(name="psum", bufs=1, space="PSUM") as psump,
        ):
            # selector rides the gpsimd SWDGE queue so the SP/ACT HWDGE
            # queues start streaming mask tiles immediately
            sel_t = constp.tile([128, NB * GSZ], fp8)
            nc.gpsimd.dma_start(sel_t[:], sel)
            rowany = constp.tile([128, NB + 1], fp32)
            nc.gpsimd.memset(rowany[:], 0.0)
            csb = constp.tile([GSZ, W], fp16)
            # one PSUM tile per 512-col chunk: csb chunk ci then depends only
            # on chain ci's stop matmul, not on all three
            cc = [
                psump.tile([GSZ, cw], fp32, name=f"cc{ci}", tag=f"cc{ci}")
                for ci, (_, cw) in enumerate(chunks)
            ]

            b_idx = 0
            n_taper = sum(1 for _, t in groups if t < max(GROUP_SIZES))
            for gi, (R, T) in enumerate(groups):
                if gi == len(groups) - 1:
                    # Final tile arrives as two half-column fetches into
                    # independent tiles: the left half binarizes and ships
                    # while the right half is still in flight, halving the
                    # serial work left after the very last byte lands.
                    assert T == 1
                    for hf, (c0, cw) in enumerate(((0, W // 2), (W // 2, W // 2))):
                        rawh = rawp.tile([128, tmax * W], fp32, tag="raw")
                        nc.sync.dma_start(
                            rawh[:P, :cw], masks[R : R + P, c0 : c0 + cw]
                        )
                        bl = constp.tile([P, W // 2], fp8, name=f"blast{hf}")
                        nc.vector.tensor_scalar(
                            out=bl[:, :],
                            in0=rawh[:P, :cw],
                            scalar1=THRESHOLD,
                            scalar2=None,
                            op0=Op.is_gt,
                            op1=Op.max,
                            accum_out=rowany[:P, b_idx + hf : b_idx + hf + 1],
                        )
                        nc.scalar.dma_start(blast_out[:, c0 : c0 + cw], bl[:, :])
                    b_idx += 1
                    continue
                raw = rawp.tile([128, tmax * W], fp32, tag="raw")
                # Bulk groups alternate the two HWDGE rings (SP/ACT) so
                # descriptor generation always overlaps a drain. The tapered
                # tail groups all ride SP: FIFO order within one ring makes
                # them complete in order, so the trailing compute pipelines
                # with the remaining transfers instead of stacking up after
                # a simultaneous round-robin finish.
                if gi >= len(groups) - n_taper:
                    eng = nc.sync
                else:
                    eng = nc.sync if gi % 2 == 0 else nc.scalar
                eng.dma_start(
                    raw[:P, : T * W],
                    masks[R : R + P * T, :].rearrange("(p a) x -> p (a x)", p=P),
                )
                for a in range(T):
                    b = binp.tile([128, W], fp8, tag="b")
                    nc.vector.tensor_scalar(
                        out=b[:P, :],
                        in0=raw[:P, a * W : (a + 1) * W],
                        scalar1=THRESHOLD,
                        scalar2=None,
                        op0=Op.is_gt,
                        op1=Op.max,
                        accum_out=rowany[:P, b_idx : b_idx + 1],
                    )
                    for ci, (c0, cw) in enumerate(chunks):
                        nc.tensor.matmul(
                            cc[ci][:, :],
                            sel_t[:P, b_idx * GSZ : b_idx * GSZ + GSZ],
                            b[:P, c0 : c0 + cw],
                            start=(b_idx == 0),
                            stop=(b_idx == NB - 2),
                        )
                    b_idx += 1

            # (count > 0) -> fp16 presence bits, flushed per 512-col chunk so
            # each chunk's convert+DMA pipelines behind that chunk's final
            # matmul instead of waiting for all three PSUM banks.
            for ci, (c0, cw) in enumerate(chunks):
                nc.vector.tensor_scalar(
                    out=csb[:, c0 : c0 + cw],
                    in0=cc[ci][:, :],
                    scalar1=0.0,
                    scalar2=None,
                    op0=Op.is_gt,
                )
                nc.sync.dma_start(counts_out[:, c0 : c0 + cw], csb[:, c0 : c0 + cw])
            # SP HWDGE (idle by now), not gpsimd: SWDGE's ~1us generation
            # would co-gate the kernel end alongside the blast path
            nc.sync.dma_start(rowany_out, rowany[:])

    nc.compile()
    return nc


def make_sel(core):
    """Per-block one-hot selector: partition p -> local mask index."""
    g = core * SHARD_ROWS + LOCAL_ROWS  # [P, NB] global rows
    first = (core * SHARD_ROWS) // H
    ul = g // H - first
    assert ul.min() >= 0 and ul.max() < GSZ
    sel = np.zeros((128, NB * GSZ), ml_dtypes.float8_e4m3)
    sel[np.arange(P)[:, None], np.arange(NB)[None, :] * GSZ + ul] = 1
    return sel


def postprocess(results):
    """Per-core rowany/counts -> boxes [N, 2, 2] f32 (exact)."""
    v1 = np.zeros(N)  # H - ymin   (0 if empty)
    v2 = np.zeros(N)  # ymax + 1
    u1 = np.zeros(N)  # W - xmin
    u2 = np.zeros(N)  # xmax + 1
    xs = np.arange(W)
    for c, r in enumerate(results):
        g = c * SHARD_ROWS + LOCAL_ROWS
        unit = g // H
        y = g % H
        ra = np.asarray(r["rowany_out"])[:P]
        a = ra[:, :NB] > 0
        # final block's two half-width binarizes live in cols NB-1 and NB
        a[:, NB - 1] = (ra[:, NB - 1] > 0) | (ra[:, NB] > 0)
        np.maximum.at(v1, unit[a], (H - y)[a])
        np.maximum.at(v2, unit[a], (y + 1)[a])
        first = (c * SHARD_ROWS) // H
        nu = ((c + 1) * SHARD_ROWS - 1) // H - first + 1
        p = np.asarray(r["counts_out"][:nu]) > 0  # [nu, W]
        np.maximum.at(u1, first + np.arange(nu), np.max(np.where(p, W - xs, 0), 1))
        np.maximum.at(u2, first + np.arange(nu), np.max(np.where(p, xs + 1, 0), 1))
        # final block bypassed the PE: fold its columns in here
        # (its y-bounds are already covered by rowany)
        blast = np.asarray(r["blast_out"]) > 0  # [P, W]
        u79 = unit[:, NB - 1]  # [P] global mask id per partition
        for uu in np.unique(u79):
            colany = blast[u79 == uu].any(0)
            u1[uu] = max(u1[uu], np.where(colany, W - xs, 0).max())
            u2[uu] = max(u2[uu], np.where(colany, xs + 1, 0).max())
    boxes = np.empty((N, 2, 2), np.float32)
    boxes[:, 0, 0] = W - u1  # xmin
    boxes[:, 0, 1] = H - v1  # ymin
    boxes[:, 1, 0] = u2 - 1  # xmax
    boxes[:, 1, 1] = v2 - 1  # ymax
    return boxes


_cache = {}


def _get_program():
    if "nc" not in _cache:
        _cache["nc"] = build_program()
        _cache["sel"] = [make_sel(c) for c in range(N_CORES)]
    return _cache["nc"], _cache["sel"]


def make_in_maps(masks):
    masks = np.ascontiguousarray(np.asarray(masks, dtype=np.float32))
    _, sels = _get_program()
    flat = masks.reshape(ROWS, W)
    return [
        {"masks": flat[c * SHARD_ROWS : (c + 1) * SHARD_ROWS], "sel": sels[c]}
        for c in range(N_CORES)
    ]


def kernel(masks):
    nc, _ = _get_program()
    in_maps = make_in_maps(masks)
    res = run_bass_kernel_spmd(nc, in_maps, core_ids=list(range(N_CORES)))
    return postprocess(res.results)
